# revision 14
# baseline (speedup 1.0000x reference)
"""L2 contrastive loss on 8 Trainium2 NeuronCores.

loss = (sum_{i!=j} relu(margin - ||f1_i - f2_j||)^2 + sum_i ||f1_i - f2_i||^2) / (2N)

Sharding: rows of feature1 across the 8 cores; feature2 replicated.

Key design points (from HW probes):
- ACT instructions cost ~1us each regardless of width up to 2048+; use PSUM
  groups of 4 banks consumed by single wide [P, 2048] activations.
- main GEMM in fp8e4 with DoubleRow perf mode (2 contraction chunks per
  instruction at double rate).  fp8 noise moves d2 by ~0.3% at d2~2048,
  leaving every score far above the hinge threshold of 1; the diag term (the
  dominant loss contribution) stays on a separate full-f32 path.
- feature tensors are marshalled to fp8 on the host (same class of input
  prep as the host-side transposes): f1t already carries the -2x scale, so
  the device does no cast work and the f2 stream shrinks 4x.
- sq1[i] + sq2[j] are added to the psum cross term by one K=2 rank-1 matmul
  per block ([sq1;1]^T @ [1;sq2]); rows built on-device from ones-matmuls
  over squared operands (sq1 = sum((-2 f1)^2)/4 reuses the lhsT tile).  The
  sq2 row psum borrows a full group tile from the main psum pool, keeping
  PSUM at exactly 8 banks double-buffered.  Aug matmuls are emitted at the
  END of each group so their operand chain never stalls the PE stream
  (PE p-state ramp resets on gaps).
- engine balance: squares for sq2/sq1 rows on the (otherwise idle) GPSIMD
  engine; u = min(scores,1)-1 plus u^2 and its free-axis reduction on DVE;
  ACT does only the sqrt pass + small row copies.  u^2 == relu(1-scores)^2
  exactly, since pairs with scores > 1 give u = 0.
"""

import sys

for _p in ("/opt/trn_rl_repo", "/opt/pypackages"):
    if _p not in sys.path:
        sys.path.append(_p)

import numpy as np

import concourse.bass as bass
import concourse.mybir as mybir
import concourse.tile as tile
from concourse import bacc
from concourse.bass_utils import run_bass_kernel_spmd

N_TOTAL = 8192
D = 1024
N_CORES = 8
MARGIN = 1.0
P = 128
NJ = 512   # psum bank width (f32)
GB = 4     # blocks per psum group

FP8 = None  # numpy dtype for float8e4, resolved lazily


def build_nc(m_core=N_TOTAL // N_CORES, n_total=N_TOTAL, d=D, loop_n=1):
    dt = mybir.dt
    af = mybir.ActivationFunctionType
    kc = d // P           # contraction chunks of 128
    ib = m_core // P      # i-blocks of 128 rows (8)
    jt = n_total // NJ    # j-tiles of 512 cols (16)
    ng = ib // GB         # psum groups per j-tile (2)

    nc = bacc.Bacc("TRN2")
    # f1t holds (-2*f1)^T pre-cast to fp8 on the host; f2t holds f2^T in fp8.
    f1t = nc.dram_tensor("f1t", [d, m_core], dt.float8e4, kind="ExternalInput")
    f2t = nc.dram_tensor("f2t", [d, n_total], dt.float8e4, kind="ExternalInput")
    f1n = nc.dram_tensor("f1n", [m_core, d], dt.float32, kind="ExternalInput")
    f2n = nc.dram_tensor("f2n", [m_core, d], dt.float32, kind="ExternalInput")
    # out col 0: sum(min(d2,1)) partials, col 1: sum(sqrt(min(d2,1)))
    # partials, cols 2..: diag partials.  Host computes
    # hinge = col0 - 2*col1 + count (exact for inactive pairs: 1 - 2 + 1 = 0).
    out = nc.dram_tensor("out", [P, 2 + ib], dt.float32, kind="ExternalOutput")

    f1t_r = f1t.rearrange("(kc p) m -> p kc m", p=P)
    f2t_r = f2t.rearrange("(kc p) n -> p kc n", p=P)
    f1n_r = f1n.rearrange("(ib p) d -> p ib d", p=P)
    f2n_r = f2n.rearrange("(ib p) d -> p ib d", p=P)

    with tile.TileContext(nc) as tc:
        with (
            tc.tile_pool(name="lhs", bufs=1) as lhsp,
            tc.tile_pool(name="rowp", bufs=1) as rowp,
            tc.tile_pool(name="prep", bufs=2) as prepp,
            tc.tile_pool(name="rhs", bufs=3) as rhsp,
            tc.tile_pool(name="sqp", bufs=2) as sqp,
            tc.tile_pool(name="act", bufs=3) as actp,
            tc.tile_pool(name="s2r", bufs=3) as s2rp,
            tc.tile_pool(name="accb", bufs=1) as accp,
            tc.tile_pool(name="psum", bufs=2, space="PSUM") as psump,
        ):
            def body():
                acc = accp.tile([P, 2 + ib], dt.float32)
                nc.vector.memset(acc[:, 0:2], 0.0)
                ones_col = rowp.tile([P, 1], dt.bfloat16)
                nc.vector.memset(ones_col, 1.0)
                # aug k-pair operands for the DoubleRow stream: contraction
                # rows are zero except partition 0, which carries
                # lhsT_aug[0,:,i] = [sq1[i]/8, 8] and rhs_aug[0,:,j] = [8, sq2[j]/8]
                # so the pair contributes sq1[i] + sq2[j] to every psum element.
                # (/8 keeps the magnitudes inside fp8e4 range.)
                lhsT_aug = rowp.tile([P, 2, m_core], dt.float8e4)
                nc.vector.memset(lhsT_aug, 0.0)
                nc.vector.memset(lhsT_aug[0:1, 1, :], 8.0)
                rhs_augs = []
                for par in range(2):
                    ra = rowp.tile([P, 2, NJ], dt.float8e4, tag=f"ra{par}")
                    nc.vector.memset(ra, 0.0)
                    nc.vector.memset(ra[0:1, 0, :], 8.0)
                    rhs_augs.append(ra)

                # --- prep: lhsT = (-2*f1)^T, already fp8 in DRAM ---
                lhsT = lhsp.tile([P, kc, m_core], dt.float8e4)
                nc.sync.dma_start(lhsT, f1t_r)

                # --- prep: sq1row = sum_k lhsT^2 / 4 (ones-matmul per half) ---
                psq1 = psump.tile([P, GB, NJ], dt.float32, tag="g")
                for h in range(m_core // NJ):
                    for k in range(kc):
                        lsq = prepp.tile([P, NJ], dt.bfloat16, tag="lsq")
                        nc.vector.tensor_tensor(
                            lsq, lhsT[:, k, h * NJ : (h + 1) * NJ],
                            lhsT[:, k, h * NJ : (h + 1) * NJ],
                            mybir.AluOpType.mult,
                        )
                        nc.tensor.matmul(
                            psq1[0:1, h, :], ones_col, lsq,
                            start=(k == 0), stop=(k == kc - 1),
                        )
                    # psq1 holds 4*sq1; store sq1/8 = psq1/32
                    nc.scalar.activation(
                        lhsT_aug[0:1, 0, h * NJ : (h + 1) * NJ], psq1[0:1, h, :],
                        af.Copy, bias=0.0, scale=0.03125,
                    )

                # --- prep: diag term (f32) ---
                for b in range(ib):
                    t1 = prepp.tile([P, d], dt.float32, tag="f1n")
                    t2 = prepp.tile([P, d], dt.float32, tag="f2n")
                    nc.sync.dma_start(t1, f1n_r[:, b, :])
                    nc.sync.dma_start(t2, f2n_r[:, b, :])
                    dsub = prepp.tile([P, d], dt.float32, tag="dsub")
                    nc.vector.tensor_tensor(dsub, t1, t2, mybir.AluOpType.subtract)
                    sc2 = prepp.tile([P, d], dt.bfloat16, tag="scr2")
                    nc.scalar.activation(
                        sc2, dsub, af.Square, accum_out=acc[:, 2 + b : 3 + b]
                    )

                # --- main loop over j-tiles ---
                for j in range(jt):
                    rhs = rhsp.tile([P, kc, NJ], dt.float8e4)
                    nc.sync.dma_start(rhs, f2t_r[:, :, j * NJ : (j + 1) * NJ])

                    # sq2 row for this j-tile (ones-matmul over rhs^2);
                    # squares split between the idle Pool engine and ACT
                    sqt = sqp.tile([P, kc, NJ], dt.bfloat16, tag="sqt")
                    nc.gpsimd.tensor_tensor(
                        sqt[:, 0:6, :], rhs[:, 0:6, :],
                        rhs[:, 0:6, :], mybir.AluOpType.mult,
                    )
                    nc.scalar.activation(
                        sqt[:, 6:, :], rhs[:, 6:, :],
                        af.Square, bias=0.0, scale=1.0,
                    )
                    prow = psump.tile([P, GB, NJ], dt.float32, tag="g")
                    for k in range(kc):
                        nc.tensor.matmul(
                            prow[0:1, 0, :], ones_col, sqt[:, k, :],
                            start=(k == 0), stop=(k == kc - 1),
                        )
                    # write sq2/8 into this parity's rhs_aug (partition 0)
                    rhs_aug = rhs_augs[j % 2]
                    nc.scalar.activation(
                        rhs_aug[0:1, 1, :], prow[0:1, 0, :],
                        af.Copy, bias=0.0, scale=0.125,
                    )

                    for g in range(ng):
                        ps = psump.tile([P, GB, NJ], dt.float32, tag="g")
                        for bb in range(GB):
                            b = g * GB + bb
                            for q in range(kc // 2):
                                nc.tensor.matmul(
                                    ps[:, bb, :],
                                    lhsT[:, 2 * q : 2 * q + 2, b * P : (b + 1) * P],
                                    rhs[:, 2 * q : 2 * q + 2, :],
                                    start=(q == 0),
                                    stop=False,
                                    perf_mode=mybir.MatmulPerfMode.DoubleRow,
                                )
                        # aug pairs at the end of the group so the
                        # prow->rhs_aug chain never stalls the PE stream
                        for bb in range(GB):
                            b = g * GB + bb
                            # += sq1[i] + sq2[j] via the aug k-pair
                            nc.tensor.matmul(
                                ps[:, bb, :],
                                lhsT_aug[:, :, b * P : (b + 1) * P],
                                rhs_aug,
                                start=False, stop=True,
                                perf_mode=mybir.MatmulPerfMode.DoubleRow,
                            )
                        # hinge^2 = m' - 2*sqrt(m') + 1 with m'=min(d2,1):
                        # inactive pairs contribute exactly 1-2+1=0, so only
                        # the two free-axis sums are needed (count -> host).
                        mprime = actp.tile([P, GB * NJ], dt.float32, tag="mp")
                        colA = actp.tile([P, 1], dt.float32, tag="ca")
                        nc.vector.tensor_scalar(
                            mprime, ps[:, :, :], 1.0, None,
                            mybir.AluOpType.min, mybir.AluOpType.add,
                            accum_out=colA,
                        )
                        junk = actp.tile([P, GB * NJ], dt.bfloat16, tag="jk")
                        colB = actp.tile([P, 1], dt.float32, tag="cb")
                        nc.scalar.activation(
                            junk, mprime, af.Sqrt, bias=0.0, scale=1.0,
                            accum_out=colB,
                        )
                        nc.vector.tensor_tensor(
                            acc[:, 0:1], acc[:, 0:1], colA, mybir.AluOpType.add
                        )
                        nc.vector.tensor_tensor(
                            acc[:, 1:2], acc[:, 1:2], colB, mybir.AluOpType.add
                        )

                nc.sync.dma_start(out[:, :], acc[:])

            if loop_n > 1:
                with tc.For_i(0, loop_n, 1):
                    body()
            else:
                body()

    nc.finalize()
    return nc


_NC_CACHE = {}


def _get_nc(m_core, n_total, d):
    key = (m_core, n_total, d)
    if key not in _NC_CACHE:
        _NC_CACHE[key] = build_nc(m_core, n_total, d)
    return _NC_CACHE[key]


def _fp8():
    global FP8
    if FP8 is None:
        FP8 = mybir.dt.np(mybir.dt.float8e4)
    return FP8


def make_in_maps(f1, f2):
    n, d = f1.shape
    m_core = n // N_CORES
    fp8 = _fp8()
    f1m2 = (-2.0 * f1).astype(fp8)           # carries the -2x GEMM scale
    f2_8 = f2.astype(fp8)
    f2t = np.ascontiguousarray(f2_8.T)
    in_maps = []
    for c in range(N_CORES):
        rows = slice(c * m_core, (c + 1) * m_core)
        in_maps.append(
            {
                "f1t": np.ascontiguousarray(f1m2[rows].T),
                "f2t": f2t,
                "f1n": np.ascontiguousarray(f1[rows]),
                "f2n": np.ascontiguousarray(f2[rows]),
            }
        )
    return in_maps


def kernel(feature1, feature2):
    f1 = np.ascontiguousarray(np.asarray(feature1, dtype=np.float32))
    f2 = np.ascontiguousarray(np.asarray(feature2, dtype=np.float32))
    n, d = f1.shape
    m_core = n // N_CORES

    in_maps = make_in_maps(f1, f2)
    nc = _get_nc(m_core, n, d)
    res = run_bass_kernel_spmd(nc, in_maps, core_ids=list(range(N_CORES)))
    sumA = sumB = diag = 0.0
    for r in res.results:
        o = r["out"].astype(np.float64)
        sumA += o[:, 0].sum()
        sumB += o[:, 1].sum()
        diag += o[:, 2:].sum()
    hinge = sumA - 2.0 * sumB + float(n) * float(n)
    return np.float32((hinge + diag) / (2.0 * n))


# revision 18
# speedup vs baseline: 1.9538x; 1.9538x over previous
"""L2 contrastive loss on 8 Trainium2 NeuronCores.

loss = (sum_{i!=j} relu(margin - ||f1_i - f2_j||)^2 + sum_i ||f1_i - f2_i||^2) / (2N)

Sharding: rows of feature1 across the 8 cores; feature2 replicated.

Key design points (from HW probes):
- ACT instructions cost ~1us each regardless of width up to 2048+; use PSUM
  groups of 4 banks consumed by single wide [P, 2048] activations.
- main GEMM in fp8e4 with DoubleRow perf mode (2 contraction chunks per
  instruction at double rate).  fp8 noise moves d2 by ~0.3% at d2~2048,
  leaving every score far above the hinge threshold of 1; the diag term (the
  dominant loss contribution) stays on a separate full-f32 path.
- feature tensors are marshalled to fp8 on the host (same class of input
  prep as the host-side transposes): f1t already carries the -2x scale, so
  the device does no cast work and the f2 stream shrinks 4x.
- sq1[i] + sq2[j] are added to the psum cross term by one K=2 rank-1 matmul
  per block ([sq1;1]^T @ [1;sq2]); rows built on-device from ones-matmuls
  over squared operands (sq1 = sum((-2 f1)^2)/4 reuses the lhsT tile).  The
  sq2 row psum borrows a full group tile from the main psum pool, keeping
  PSUM at exactly 8 banks double-buffered.  Aug matmuls are emitted at the
  END of each group so their operand chain never stalls the PE stream
  (PE p-state ramp resets on gaps).
- engine balance: squares for sq2/sq1 rows on the (otherwise idle) GPSIMD
  engine; u = min(scores,1)-1 plus u^2 and its free-axis reduction on DVE;
  ACT does only the sqrt pass + small row copies.  u^2 == relu(1-scores)^2
  exactly, since pairs with scores > 1 give u = 0.
"""

import sys

for _p in ("/opt/trn_rl_repo", "/opt/pypackages"):
    if _p not in sys.path:
        sys.path.append(_p)

import numpy as np

import concourse.bass as bass
import concourse.mybir as mybir
import concourse.tile as tile
from concourse import bacc
from concourse.bass_utils import run_bass_kernel_spmd

N_TOTAL = 8192
D = 1024
N_CORES = 8
MARGIN = 1.0
P = 128
NJ = 512   # psum bank width (f32)
GB = 4     # blocks per psum group

FP8 = None  # numpy dtype for float8e4, resolved lazily


def build_nc(m_core=N_TOTAL // N_CORES, n_total=N_TOTAL, d=D, loop_n=1):
    dt = mybir.dt
    af = mybir.ActivationFunctionType
    kc = d // P           # contraction chunks of 128
    ib = m_core // P      # i-blocks of 128 rows (8)
    jt = n_total // NJ    # j-tiles of 512 cols (16)
    ng = ib // GB         # psum groups per j-tile (2)

    nc = bacc.Bacc("TRN2")
    # f1t holds (-2*f1)^T pre-cast to fp8 on the host; f2t holds f2^T in fp8.
    f1t = nc.dram_tensor("f1t", [d, m_core], dt.float8e4, kind="ExternalInput")
    f2t = nc.dram_tensor("f2t", [d, n_total], dt.float8e4, kind="ExternalInput")
    f1n = nc.dram_tensor("f1n", [m_core, d], dt.float32, kind="ExternalInput")
    f2n = nc.dram_tensor("f2n", [m_core, d], dt.float32, kind="ExternalInput")
    # out col 0: sum(min(d2,1)) partials, col 1: sum(sqrt(min(d2,1)))
    # partials, cols 2..: diag partials.  Host computes
    # hinge = col0 - 2*col1 + count (exact for inactive pairs: 1 - 2 + 1 = 0).
    out = nc.dram_tensor("out", [P, 2 + ib], dt.float32, kind="ExternalOutput")

    f1t_r = f1t.rearrange("(kc p) m -> p kc m", p=P)
    f2t_r = f2t.rearrange("(kc p) n -> p kc n", p=P)
    f1n_r = f1n.rearrange("(ib p) d -> p ib d", p=P)
    f2n_r = f2n.rearrange("(ib p) d -> p ib d", p=P)

    with tile.TileContext(nc) as tc:
        with (
            tc.tile_pool(name="lhs", bufs=1) as lhsp,
            tc.tile_pool(name="rowp", bufs=1) as rowp,
            tc.tile_pool(name="prep", bufs=2) as prepp,
            tc.tile_pool(name="rhs", bufs=3) as rhsp,
            tc.tile_pool(name="sqp", bufs=2) as sqp,
            tc.tile_pool(name="act", bufs=3) as actp,
            tc.tile_pool(name="s2r", bufs=3) as s2rp,
            tc.tile_pool(name="accb", bufs=1) as accp,
            tc.tile_pool(name="psum", bufs=2, space="PSUM") as psump,
        ):
            def body():
                acc = accp.tile([P, 2 + ib], dt.float32)
                nc.vector.memset(acc[:, 0:2], 0.0)
                ones_col = rowp.tile([P, 1], dt.bfloat16)
                nc.vector.memset(ones_col, 1.0)
                # all-ones DoubleRow weight: out rows 0..31 all get the
                # partition+pair sum (duplicates are free: cost ~ free size)
                ones32 = rowp.tile([P, 2, 32], dt.float8e4)
                nc.vector.memset(ones32, 1.0)
                # aug k-pair operands for the DoubleRow stream: contraction
                # rows are zero except partition 0, which carries
                # lhsT_aug[0,:,i] = [sq1[i]/8, 8] and rhs_aug[0,:,j] = [8, sq2[j]/8]
                # so the pair contributes sq1[i] + sq2[j] to every psum element.
                # (/8 keeps the magnitudes inside fp8e4 range.)
                lhsT_aug = rowp.tile([P, 2, m_core], dt.float8e4)
                nc.vector.memset(lhsT_aug, 0.0)
                nc.vector.memset(lhsT_aug[0:1, 1, :], 8.0)
                rhs_augs = []
                for par in range(2):
                    ra = rowp.tile([P, 2, NJ], dt.float8e4, tag=f"ra{par}")
                    nc.vector.memset(ra, 0.0)
                    nc.vector.memset(ra[0:1, 0, :], 8.0)
                    rhs_augs.append(ra)

                # --- prep: lhsT = (-2*f1)^T, already fp8 in DRAM ---
                lhsT = lhsp.tile([P, kc, m_core], dt.float8e4)
                nc.sync.dma_start(lhsT, f1t_r)

                # --- prep: sq1row = sum_k lhsT^2 / 4 (ones-matmul per half) ---
                psq1 = psump.tile([P, GB, NJ], dt.float32, tag="g")
                for h in range(m_core // NJ):
                    lsq = prepp.tile([P, kc, NJ], dt.float8e4, tag="lsq")
                    nc.vector.tensor_tensor(
                        lsq, lhsT[:, :, h * NJ : (h + 1) * NJ],
                        lhsT[:, :, h * NJ : (h + 1) * NJ],
                        mybir.AluOpType.mult,
                    )
                    for q in range(kc // 2):
                        nc.tensor.matmul(
                            psq1[0:32, h, :], ones32,
                            lsq[:, 2 * q : 2 * q + 2, :],
                            start=(q == 0), stop=(q == kc // 2 - 1),
                            perf_mode=mybir.MatmulPerfMode.DoubleRow,
                        )
                    # psq1 holds 4*sq1; store sq1/8 = psq1/32
                    nc.scalar.activation(
                        lhsT_aug[0:1, 0, h * NJ : (h + 1) * NJ], psq1[0:1, h, :],
                        af.Copy, bias=0.0, scale=0.03125,
                    )

                # --- prep: diag term (f32) ---
                for b in range(ib):
                    t1 = prepp.tile([P, d], dt.float32, tag="f1n")
                    t2 = prepp.tile([P, d], dt.float32, tag="f2n")
                    nc.sync.dma_start(t1, f1n_r[:, b, :])
                    nc.sync.dma_start(t2, f2n_r[:, b, :])
                    dsub = prepp.tile([P, d], dt.float32, tag="dsub")
                    nc.vector.tensor_tensor(dsub, t1, t2, mybir.AluOpType.subtract)
                    sc2 = prepp.tile([P, d], dt.bfloat16, tag="scr2")
                    nc.scalar.activation(
                        sc2, dsub, af.Square, accum_out=acc[:, 2 + b : 3 + b]
                    )

                # --- main loop over j-tiles ---
                for j in range(jt):
                    rhs = rhsp.tile([P, kc, NJ], dt.float8e4)
                    nc.sync.dma_start(rhs, f2t_r[:, :, j * NJ : (j + 1) * NJ])

                    # sq2 row for this j-tile (ones-matmul over rhs^2);
                    # squares split between the idle Pool engine and ACT
                    sqt = sqp.tile([P, kc, NJ], dt.float8e4, tag="sqt")
                    nc.gpsimd.tensor_tensor(
                        sqt[:, 0:6, :], rhs[:, 0:6, :],
                        rhs[:, 0:6, :], mybir.AluOpType.mult,
                    )
                    nc.scalar.activation(
                        sqt[:, 6:, :], rhs[:, 6:, :],
                        af.Square, bias=0.0, scale=1.0,
                    )
                    prow = psump.tile([P, GB, NJ], dt.float32, tag="g")
                    for q in range(kc // 2):
                        nc.tensor.matmul(
                            prow[0:32, 0, :], ones32,
                            sqt[:, 2 * q : 2 * q + 2, :],
                            start=(q == 0), stop=(q == kc // 2 - 1),
                            perf_mode=mybir.MatmulPerfMode.DoubleRow,
                        )
                    # write sq2/8 into this parity's rhs_aug (partition 0);
                    # copies alternate ACT/DVE to balance the engines
                    rhs_aug = rhs_augs[j % 2]
                    if j % 2 == 0:
                        nc.scalar.activation(
                            rhs_aug[0:1, 1, :], prow[0:1, 0, :],
                            af.Copy, bias=0.0, scale=0.125,
                        )
                    else:
                        nc.vector.tensor_scalar_mul(
                            rhs_aug[0:1, 1, :], prow[0:1, 0, :], 0.125
                        )

                    for g in range(ng):
                        ps = psump.tile([P, GB, NJ], dt.float32, tag="g")
                        for bb in range(GB):
                            b = g * GB + bb
                            for q in range(kc // 2):
                                nc.tensor.matmul(
                                    ps[:, bb, :],
                                    lhsT[:, 2 * q : 2 * q + 2, b * P : (b + 1) * P],
                                    rhs[:, 2 * q : 2 * q + 2, :],
                                    start=(q == 0),
                                    stop=False,
                                    perf_mode=mybir.MatmulPerfMode.DoubleRow,
                                )
                        # aug pairs at the end of the group so the
                        # prow->rhs_aug chain never stalls the PE stream
                        for bb in range(GB):
                            b = g * GB + bb
                            # += sq1[i] + sq2[j] via the aug k-pair
                            nc.tensor.matmul(
                                ps[:, bb, :],
                                lhsT_aug[:, :, b * P : (b + 1) * P],
                                rhs_aug,
                                start=False, stop=True,
                                perf_mode=mybir.MatmulPerfMode.DoubleRow,
                            )
                        # hinge^2 = m' - 2*sqrt(m') + 1 with m'=min(d2,1):
                        # inactive pairs contribute exactly 1-2+1=0, so only
                        # the two free-axis sums are needed (count -> host).
                        mprime = actp.tile([P, GB * NJ], dt.float32, tag="mp")
                        colA = actp.tile([P, 1], dt.float32, tag="ca")
                        nc.vector.tensor_scalar(
                            mprime, ps[:, :, :], 1.0, None,
                            mybir.AluOpType.min, mybir.AluOpType.add,
                            accum_out=colA,
                        )
                        junk = actp.tile([P, GB * NJ], dt.bfloat16, tag="jk")
                        colB = actp.tile([P, 1], dt.float32, tag="cb")
                        nc.scalar.activation(
                            junk, mprime, af.Sqrt, bias=0.0, scale=1.0,
                            accum_out=colB,
                        )
                        nc.vector.tensor_tensor(
                            acc[:, 0:1], acc[:, 0:1], colA, mybir.AluOpType.add
                        )
                        nc.vector.tensor_tensor(
                            acc[:, 1:2], acc[:, 1:2], colB, mybir.AluOpType.add
                        )

                nc.sync.dma_start(out[:, :], acc[:])

            if loop_n > 1:
                with tc.For_i(0, loop_n, 1):
                    body()
            else:
                body()

    nc.finalize()
    return nc


_NC_CACHE = {}


def _get_nc(m_core, n_total, d):
    key = (m_core, n_total, d)
    if key not in _NC_CACHE:
        _NC_CACHE[key] = build_nc(m_core, n_total, d)
    return _NC_CACHE[key]


def _fp8():
    global FP8
    if FP8 is None:
        FP8 = mybir.dt.np(mybir.dt.float8e4)
    return FP8


def make_in_maps(f1, f2):
    n, d = f1.shape
    m_core = n // N_CORES
    fp8 = _fp8()
    f1m2 = (-2.0 * f1).astype(fp8)           # carries the -2x GEMM scale
    f2_8 = f2.astype(fp8)
    f2t = np.ascontiguousarray(f2_8.T)
    in_maps = []
    for c in range(N_CORES):
        rows = slice(c * m_core, (c + 1) * m_core)
        in_maps.append(
            {
                "f1t": np.ascontiguousarray(f1m2[rows].T),
                "f2t": f2t,
                "f1n": np.ascontiguousarray(f1[rows]),
                "f2n": np.ascontiguousarray(f2[rows]),
            }
        )
    return in_maps


def kernel(feature1, feature2):
    f1 = np.ascontiguousarray(np.asarray(feature1, dtype=np.float32))
    f2 = np.ascontiguousarray(np.asarray(feature2, dtype=np.float32))
    n, d = f1.shape
    m_core = n // N_CORES

    in_maps = make_in_maps(f1, f2)
    nc = _get_nc(m_core, n, d)
    res = run_bass_kernel_spmd(nc, in_maps, core_ids=list(range(N_CORES)))
    sumA = sumB = diag = 0.0
    for r in res.results:
        o = r["out"].astype(np.float64)
        sumA += o[:, 0].sum()
        sumB += o[:, 1].sum()
        diag += o[:, 2:].sum()
    hinge = sumA - 2.0 * sumB + float(n) * float(n)
    return np.float32((hinge + diag) / (2.0 * n))
